# revision 57
# baseline (speedup 1.0000x reference)
"""Causal self-attention Trainium2 kernel (B=128, T=128, C=768, H=12, D=64).

Sharding: data-parallel over batch across 8 cores (16 batches/core).
Per-core pipeline (4-batch groups, fp16 matmuls everywhere):
  x (fp16, cast on host) -> x^T via XBAR DMA transpose, prefetched a group ahead
  Q^T,K^T = W_qkv^T @ x^T  (feature-major, N=512)  K^T -> zero-padded kz
  V       = x^T.T @ W_qkv[:,v] (token-major, N=384) -> V' with ones col
  S^T_h   = kz_h.T @ Q^T (fp16, K=128)
  E^T     = exp(S^T * scale) (Act) * causal01 (DVE)   [multiplicative mask]
  O'_h    = E^T_h.T @ V'_h  (N=65: cols 0:64=O', col 64=rowsum)
  O       = O' * (1/rowsum)  (DVE, token-major fp16)
  O^T via PE transposes -> Y = O^T.T @ W_proj16 -> fp16 y_sb -> DRAM
Attention/output units (b,dpr) software-pipelined (lag/lag_y) so PE never
waits on the Act-exp -> DVE-mask chain or the O^T transpose copies.
PSUM->SBUF copies balanced across Act and DVE (Pool/GPSIMD cannot read PSUM).
Default (BEST) config is the flat cross-group pipeline: QKV/V matmuls of
group g+1 are interleaved into group g's attention/output steps, giving the
PE large filler matmuls between every latency-sensitive unit and removing
per-group pipeline drains. Ring depths are chosen so per-body tile-call
counts divide bufs (required for For_i slot aliasing; qT bufs=4 also breaks
a scheduler deadlock cycle between the Act queue and PE).
"""
import sys
import numpy as np

sys.path.insert(0, "/opt/trn_rl_repo")

import concourse.bass as bass  # noqa: E402
import concourse.tile as tile  # noqa: E402
from concourse import bacc, mybir  # noqa: E402
from concourse import bass_utils  # noqa: E402
from contextlib import ExitStack, nullcontext  # noqa: E402

F32 = mybir.dt.float32
F16 = mybir.dt.float16

N_CORES = 8
B, T, C = 128, 128, 768
H, D = 12, 64
BC = B // N_CORES          # batches per core = 16
GB = 4                     # batches per group
NG = BC // GB              # groups per core = 4
GT = GB * T                # tokens per group = 512
NK = C // 128              # contraction k-tiles = 6
SCALE = D ** -0.5
LAG = 6                    # attention software-pipeline depth (S -> EV)
LAG_Y = 6                  # further lag from o_sb complete to Y matmuls
BEST = dict(flat=True, lag=LAG, lag_y=LAG_Y)   # chosen config


def build_program(loop_iters=None, py_iters=1, ot_xbar=False, lag=LAG,
                  lag_y=LAG_Y, vp_eng="act", kz_eng="split", y_eng="dve",
                  flat=False, et_bufs=6, eraw_bufs=3):
    nc = bacc.Bacc("TRN2", target_bir_lowering=False, debug=False,
                   num_devices=N_CORES)
    x_d = nc.dram_tensor("x16", [BC, T, C], F16, kind="ExternalInput").ap()
    wqkv16_d = nc.dram_tensor("w_qkv16", [C, 3 * C], F16, kind="ExternalInput").ap()
    wproj16_d = nc.dram_tensor("w_proj16", [C, C], F16, kind="ExternalInput").ap()
    mask01_d = nc.dram_tensor("mask01", [128, T], F16, kind="ExternalInput").ap()
    ident16_d = nc.dram_tensor("ident16", [128, 128], F16, kind="ExternalInput").ap()
    y_d = nc.dram_tensor("y", [BC, T, C], F16, kind="ExternalOutput").ap()

    with tile.TileContext(nc) as tc, ExitStack() as ctx:
        cpool = ctx.enter_context(tc.tile_pool(name="const", bufs=1))
        gpool = ctx.enter_context(tc.tile_pool(name="grp", bufs=2))
        spool = ctx.enter_context(tc.tile_pool(name="small", bufs=4))
        pp = ctx.enter_context(tc.tile_pool(name="ps", bufs=1, space="PSUM"))

        # ---- constants / weights (resident) ----
        wqkv16 = cpool.tile([128, NK, 3 * C], F16)
        nc.gpsimd.dma_start(wqkv16, wqkv16_d.rearrange("(k p) f -> p k f", p=128))
        wproj16 = cpool.tile([128, NK, C], F16)
        nc.gpsimd.dma_start(wproj16, wproj16_d.rearrange("(k p) f -> p k f", p=128))
        mask01 = cpool.tile([128, T], F16)
        nc.sync.dma_start(mask01, mask01_d)
        ident16 = cpool.tile([128, 128], F16)
        nc.sync.dma_start(ident16, ident16_d)

        # persistent kz / vp (ping-pong): zero halves and ones cols written once
        kz_pp = [cpool.tile([128, H, GT], F16, name=f"kz{i}") for i in range(2)]
        vp_pp = [cpool.tile([128, GB, H, 65], F16, name=f"vp{i}") for i in range(2)]
        for kzt in kz_pp:
            nc.gpsimd.memset(kzt[64:128, 0:H:2, :], 0.0)
            nc.gpsimd.memset(kzt[0:64, 1:H:2, :], 0.0)
        for vpt in vp_pp:
            nc.gpsimd.memset(vpt[:, :, :, 64:65], 1.0)

        def emit_x_chain(g):
            """Load + transpose x (fp16 from host) for group g."""
            xT = gpool.tile([128, NK, GB, 128], F16, tag="xT", bufs=4,
                             name=f"xT_{g}")
            for b in range(GB):
                x_sb = gpool.tile([128, C], F16, tag="x_sb", bufs=8,
                                  name=f"x_sb_{g}_{b}")
                nc.gpsimd.dma_start(x_sb, x_d[g * GB + b])
                nc.sync.dma_start_transpose(xT[:, :, b, :], x_sb)
            return xT

        def emit_y(g, b, oT):
            """Y projection for batch b of group g, reading transposed oT."""
            y_sb = spool.tile([128, C], F16, tag="y_sb", bufs=4, name="y_sb")
            for half in range(2):
                y_ps = pp.tile([128, 384], F32, tag="vps", bufs=2, name="y_ps")
                for k in range(NK):
                    nc.tensor.matmul(y_ps, oT[:, k, :],
                                     wproj16[:, k, 384 * half:384 * (half + 1)],
                                     start=(k == 0), stop=(k == NK - 1))
                y_copy = (nc.vector.tensor_copy if y_eng == "dve"
                          else nc.scalar.copy)
                y_copy(y_sb[:, 384 * half:384 * (half + 1)], y_ps)
            nc.sync.dma_start(y_d[g * GB + b], y_sb)

        F_ORDER = (6, 0, 7, 1, 8, 2, 9, 3, 10, 4, 11, 5)

        def emit_qk_unit(qT, kz, xg, f):
            qk_ps = pp.tile([128, GT], F32, tag="big", bufs=4, name="qk_ps")
            for k in range(NK):
                nc.tensor.matmul(qk_ps, wqkv16[:, k, 128 * f:128 * (f + 1)],
                                 xg[:, k, :], start=(k == 0), stop=(k == NK - 1))
            if f < 6:
                nc.scalar.copy(qT[:, f, :], qk_ps)
            else:
                h0 = 2 * (f - 6)
                # split the half-copies between DVE and Act
                use_dve = kz_eng == "dve" or (kz_eng == "split" and f % 2 == 0)
                if use_dve:
                    nc.vector.tensor_copy(kz[0:64, h0, :], qk_ps[0:64, :])
                    nc.vector.tensor_copy(kz[64:128, h0 + 1, :],
                                          qk_ps[64:128, :])
                else:
                    nc.scalar.copy(kz[0:64, h0, :], qk_ps[0:64, :])
                    nc.scalar.copy(kz[64:128, h0 + 1, :], qk_ps[64:128, :])

        def emit_v_unit(vp, xT, unit):
            b, half = divmod(unit, 2)
            v_ps = pp.tile([128, 384], F32, tag="vps", bufs=2, name="v_ps")
            for k in range(NK):
                nc.tensor.matmul(
                    v_ps, xT[:, k, b, :],
                    wqkv16[:, k, 2 * C + 384 * half:2 * C + 384 * (half + 1)],
                    start=(k == 0), stop=(k == NK - 1))
            vp_copy = (nc.scalar.copy if vp_eng == "act"
                       else nc.vector.tensor_copy)
            vp_copy(vp[:, b, 6 * half:6 * (half + 1), 0:64],
                    v_ps.rearrange("p (h d) -> p h d", d=64))

        def new_qT(g):
            return gpool.tile([128, 6, GT], F16, tag="qT", bufs=4,
                              name=f"qT_{g}")

        # prologue: first group's x-chain (steady state comes from the loop tail)
        xT_next = emit_x_chain(0)
        if flat:
            # flat mode: qk/V of group g+1 are interleaved into group g's
            # attention pipeline, so prefetch runs two groups ahead.
            xT_cur = xT_next
            xT_next = emit_x_chain(1)
            qT_next = new_qT(0)
            xg0 = xT_cur.rearrange("p k b t -> p k (b t)")
            for f in F_ORDER:
                emit_qk_unit(qT_next, kz_pp[0], xg0, f)
            for unit in range(2 * GB):
                emit_v_unit(vp_pp[0], xT_cur, unit)

        loop_cm = tc.For_i(0, loop_iters, 1) if loop_iters else nullcontext()
        with loop_cm:
          for _rep in range(py_iters):
            for g in range(NG):
              if flat:
                # qT/kz/vp for g were produced during g-1 (or the prologue);
                # xT_next holds x^T(g+1) for the interleaved qk/V below.
                qT, kz, vp = qT_next, kz_pp[g % 2], vp_pp[g % 2]
                xT_np1 = xT_next
                xT_next = emit_x_chain((g + 2) % NG)
                qT_next = new_qT((g + 1) % NG)
                kz_next = kz_pp[(g + 1) % 2]
                vp_next = vp_pp[(g + 1) % 2]
                xg_np1 = xT_np1.rearrange("p k b t -> p k (b t)")

                o_sb = gpool.tile([128, GB, C], F16, tag="o_sb", name=f"o_sb_{g}")
                eTs = [None] * 12
                oTs = [None] * GB
                for step in range(max(12 + 2 * GB, 12 + lag + lag_y + 1)):
                    if step < 12:
                        emit_qk_unit(qT_next, kz_next, xg_np1, F_ORDER[step])
                    v = step - lag
                    if 0 <= v < 12:
                        b, dpr = divmod(v, 3)
                        h0 = 4 * dpr
                        eT = eTs[v]
                        op_ps = pp.tile([128, 4, 65], F32, tag="op", bufs=2,
                                        name="op_ps")
                        for j in range(4):
                            nc.tensor.matmul(op_ps[:, j, :], eT[:, j, :],
                                             vp[:, b, h0 + j, :],
                                             start=(j == 0), stop=(j == 3))
                        rinv = spool.tile([128, 4], F32, tag="rinv", name="rinv")
                        nc.vector.reciprocal(rinv, op_ps[:, :, 64])
                        nc.vector.tensor_tensor(
                            out=o_sb[:, b, h0 * D:(h0 + 4) * D].rearrange(
                                "p (h d) -> p h d", h=4),
                            in0=op_ps[:, :, 0:64],
                            in1=rinv[:, :, None].broadcast_to([128, 4, 64]),
                            op=mybir.AluOpType.mult)
                        if dpr == 2:
                            oT = spool.tile([128, NK, 128], F16, tag="oT",
                                            bufs=4, name="oT")
                            for hf in range(2):
                                ot_ps = pp.tile([128, 3, 128], F16, tag="op",
                                                bufs=2, name="ot_ps")
                                for k in range(3):
                                    kk = 3 * hf + k
                                    nc.tensor.transpose(
                                        ot_ps[:, k, :],
                                        o_sb[:, b, 128 * kk:128 * (kk + 1)],
                                        ident16)
                                nc.vector.tensor_copy(
                                    oT[:, 3 * hf:3 * hf + 3, :], ot_ps)
                            oTs[b] = oT
                    if step < 12:
                        b, dpr = divmod(step, 3)
                        h0 = 4 * dpr
                        bs = slice(b * T, (b + 1) * T)
                        st_ps = pp.tile([128, 4, T], F32, tag="big", bufs=4,
                                        name="st_ps")
                        for j in range(4):
                            nc.tensor.matmul(st_ps[:, j, :], kz[:, h0 + j, bs],
                                             qT[:, 2 * dpr + j // 2, bs],
                                             start=(j == 0), stop=(j == 3))
                        e_raw = spool.tile([128, 4, T], F16, tag="e_raw",
                                           bufs=eraw_bufs, name="e_raw")
                        nc.scalar.activation(e_raw, st_ps,
                                             mybir.ActivationFunctionType.Exp,
                                             scale=SCALE)
                        eT = spool.tile([128, 4, T], F16, tag="eT",
                                        bufs=et_bufs, name="eT")
                        nc.vector.tensor_tensor(
                            out=eT, in0=e_raw,
                            in1=mask01[:, None, :].broadcast_to([128, 4, T]),
                            op=mybir.AluOpType.mult)
                        eTs[step] = eT
                    if 12 <= step < 12 + 2 * GB:
                        emit_v_unit(vp_next, xT_np1, step - 12)
                    w = step - lag - lag_y
                    if 0 <= w < 12 and w % 3 == 2:
                        emit_y(g, w // 3, oTs[w // 3])
                continue
              else:
                xT = xT_next
                # prefetch next group's x -> fp16 -> x^T (wraps to g=0 for the
                # next loop iteration; reloads the same data, which is benign)
                xT_next = emit_x_chain((g + 1) % NG)

                # ---- Q^T / K^T projection (feature-major, fp16, N=512) ----
                qT = new_qT(g)
                kz = kz_pp[g % 2]
                xg = xT.rearrange("p k b t -> p k (b t)")
                # interleave K (f>=6) and Q (f<6) so attention can start early
                for f in F_ORDER:
                    emit_qk_unit(qT, kz, xg, f)

                # ---- V projection (token-major, fp16, N=384) into V' ----
                vp = vp_pp[g % 2]
                for unit in range(2 * GB):
                    emit_v_unit(vp, xT, unit)

                # ---- attention + output, software-pipelined (12 units) ----
                o_sb = gpool.tile([128, GB, C], F16, tag="o_sb", name=f"o_sb_{g}")
                eTs = [None] * 12
                oTs = [None] * GB
                for step in range(12 + lag + lag_y + 1):
                    v = step - lag
                    if 0 <= v < 12:
                        # EV first: frees the eT ring slot the mask below reuses
                        b, dpr = divmod(v, 3)
                        h0 = 4 * dpr
                        eT = eTs[v]
                        op_ps = pp.tile([128, 4, 65], F32, tag="op", bufs=2,
                                        name="op_ps")
                        for j in range(4):
                            nc.tensor.matmul(op_ps[:, j, :],
                                             eT[:, j, :],
                                             vp[:, b, h0 + j, :],
                                             start=(j == 0), stop=(j == 3))
                        rinv = spool.tile([128, 4], F32, tag="rinv", name="rinv")
                        nc.vector.reciprocal(rinv, op_ps[:, :, 64])
                        nc.vector.tensor_tensor(
                            out=o_sb[:, b, h0 * D:(h0 + 4) * D].rearrange(
                                "p (h d) -> p h d", h=4),
                            in0=op_ps[:, :, 0:64],
                            in1=rinv[:, :, None].broadcast_to([128, 4, 64]),
                            op=mybir.AluOpType.mult)
                        if dpr == 2:
                            oT = spool.tile([128, NK, 128], F16, tag="oT",
                                            bufs=4, name="oT")
                            if ot_xbar:
                                nc.sync.dma_start_transpose(oT, o_sb[:, b, :])
                            else:
                                # O^T via PE transposes (short latency)
                                for hf in range(2):
                                    ot_ps = pp.tile([128, 3, 128], F16, tag="op",
                                                    bufs=2, name="ot_ps")
                                    for k in range(3):
                                        kk = 3 * hf + k
                                        nc.tensor.transpose(
                                            ot_ps[:, k, :],
                                            o_sb[:, b, 128 * kk:128 * (kk + 1)],
                                            ident16)
                                    nc.vector.tensor_copy(
                                        oT[:, 3 * hf:3 * hf + 3, :], ot_ps)
                            oTs[b] = oT
                    if step < 12:
                        b, dpr = divmod(step, 3)
                        h0 = 4 * dpr
                        bs = slice(b * T, (b + 1) * T)
                        st_ps = pp.tile([128, 4, T], F32, tag="big", bufs=4,
                                        name="st_ps")
                        for j in range(4):
                            nc.tensor.matmul(st_ps[:, j, :], kz[:, h0 + j, bs],
                                             qT[:, 2 * dpr + j // 2, bs],
                                             start=(j == 0), stop=(j == 3))
                        e_raw = spool.tile([128, 4, T], F16, tag="e_raw",
                                           bufs=eraw_bufs, name="e_raw")
                        nc.scalar.activation(e_raw, st_ps,
                                             mybir.ActivationFunctionType.Exp,
                                             scale=SCALE)
                        eT = spool.tile([128, 4, T], F16, tag="eT",
                                        bufs=et_bufs, name="eT")
                        nc.vector.tensor_tensor(
                            out=eT, in0=e_raw,
                            in1=mask01[:, None, :].broadcast_to([128, 4, T]),
                            op=mybir.AluOpType.mult)
                        eTs[step] = eT
                    # Y once the oT copies have had LAG_Y units to land
                    w = step - lag - lag_y
                    if 0 <= w < 12 and w % 3 == 2:
                        bb = w // 3
                        emit_y(g, bb, oTs[bb])

    nc.compile()
    return nc


_PROGRAM = None
_in_maps_cache = None


def _host_consts():
    # S^T layout: partition = key, column = query -> keep (key <= query)
    mask01 = np.where(np.arange(T)[None, :] >= np.arange(128)[:, None],
                      np.float16(1.0), np.float16(0.0)).astype(np.float16)
    ident16 = np.eye(128, dtype=np.float16)
    return mask01, ident16


def make_in_maps(x, w_qkv, w_proj):
    x16 = np.ascontiguousarray(np.asarray(x), dtype=np.float16)
    w_qkv16 = np.ascontiguousarray(np.asarray(w_qkv), dtype=np.float16)
    w_proj16 = np.ascontiguousarray(np.asarray(w_proj), dtype=np.float16)
    mask01, ident16 = _host_consts()
    in_maps = []
    for c in range(N_CORES):
        in_maps.append({
            "x16": x16[c * BC:(c + 1) * BC],
            "w_qkv16": w_qkv16,
            "w_proj16": w_proj16,
            "mask01": mask01,
            "ident16": ident16,
        })
    return in_maps


def kernel(x, w_qkv, w_proj):
    global _PROGRAM, _in_maps_cache
    if _PROGRAM is None:
        _PROGRAM = build_program(**BEST)
    nc = _PROGRAM
    in_maps = make_in_maps(x, w_qkv, w_proj)
    _in_maps_cache = in_maps
    res = bass_utils.run_bass_kernel_spmd(nc, in_maps, core_ids=list(range(N_CORES)))
    out = np.concatenate([r["y"] for r in res.results], axis=0)
    return out.astype(np.float32)


# revision 58
# speedup vs baseline: 3.0120x; 3.0120x over previous
"""Causal self-attention Trainium2 kernel (B=128, T=128, C=768, H=12, D=64).

Sharding: data-parallel over batch across 8 cores (16 batches/core).
Per-core pipeline (4-batch groups, fp16 matmuls everywhere):
  x (fp16, cast on host) -> x^T via XBAR DMA transpose, prefetched a group ahead
  Q^T,K^T = W_qkv^T @ x^T  (feature-major, N=512)  K^T -> zero-padded kz
  V       = x^T.T @ W_qkv[:,v] (token-major, N=384) -> V' with ones col
  S^T_h   = kz_h.T @ Q^T (fp16, K=128)
  E^T     = exp(S^T * scale) (Act) * causal01 (DVE)   [multiplicative mask]
  O'_h    = E^T_h.T @ V'_h  (N=65: cols 0:64=O', col 64=rowsum)
  O       = O' * (1/rowsum)  (DVE, token-major fp16)
  O^T via PE transposes -> Y = O^T.T @ W_proj16 -> fp16 y_sb -> DRAM
Attention/output units (b,dpr) software-pipelined (lag/lag_y) so PE never
waits on the Act-exp -> DVE-mask chain or the O^T transpose copies.
PSUM->SBUF copies balanced across Act and DVE (Pool/GPSIMD cannot read PSUM).
Default (BEST) config is the flat cross-group pipeline: QKV/V matmuls of
group g+1 are interleaved into group g's attention/output steps, giving the
PE large filler matmuls between every latency-sensitive unit and removing
per-group pipeline drains. Ring depths are chosen so per-body tile-call
counts divide bufs (required for For_i slot aliasing; qT bufs=4 also breaks
a scheduler deadlock cycle between the Act queue and PE).
"""
import sys
import numpy as np

sys.path.insert(0, "/opt/trn_rl_repo")

import concourse.bass as bass  # noqa: E402
import concourse.tile as tile  # noqa: E402
from concourse import bacc, mybir  # noqa: E402
from concourse import bass_utils  # noqa: E402
from contextlib import ExitStack, nullcontext  # noqa: E402

F32 = mybir.dt.float32
F16 = mybir.dt.float16

N_CORES = 8
B, T, C = 128, 128, 768
H, D = 12, 64
BC = B // N_CORES          # batches per core = 16
GB = 4                     # batches per group
NG = BC // GB              # groups per core = 4
GT = GB * T                # tokens per group = 512
NK = C // 128              # contraction k-tiles = 6
SCALE = D ** -0.5
LAG = 6                    # attention software-pipeline depth (S -> EV)
LAG_Y = 6                  # further lag from o_sb complete to Y matmuls
BEST = dict(flat=True, lag=LAG, lag_y=LAG_Y)   # chosen config


def build_program(loop_iters=None, py_iters=1, ot_xbar=False, lag=LAG,
                  lag_y=LAG_Y, vp_eng="act", kz_eng="split", y_eng="dve",
                  flat=False, et_bufs=6, eraw_bufs=3):
    nc = bacc.Bacc("TRN2", target_bir_lowering=False, debug=False,
                   num_devices=N_CORES)
    x_d = nc.dram_tensor("x16", [BC, T, C], F16, kind="ExternalInput").ap()
    wqkv16_d = nc.dram_tensor("w_qkv16", [C, 3 * C], F16, kind="ExternalInput").ap()
    wproj16_d = nc.dram_tensor("w_proj16", [C, C], F16, kind="ExternalInput").ap()
    mask01_d = nc.dram_tensor("mask01", [128, T], F16, kind="ExternalInput").ap()
    ident16_d = nc.dram_tensor("ident16", [128, 128], F16, kind="ExternalInput").ap()
    y_d = nc.dram_tensor("y", [BC, T, C], F16, kind="ExternalOutput").ap()

    with tile.TileContext(nc) as tc, ExitStack() as ctx:
        cpool = ctx.enter_context(tc.tile_pool(name="const", bufs=1))
        gpool = ctx.enter_context(tc.tile_pool(name="grp", bufs=2))
        spool = ctx.enter_context(tc.tile_pool(name="small", bufs=4))
        pp = ctx.enter_context(tc.tile_pool(name="ps", bufs=1, space="PSUM"))

        # ---- constants / weights (resident) ----
        wqkv16 = cpool.tile([128, NK, 3 * C], F16)
        nc.gpsimd.dma_start(wqkv16, wqkv16_d.rearrange("(k p) f -> p k f", p=128))
        wproj16 = cpool.tile([128, NK, C], F16)
        nc.gpsimd.dma_start(wproj16, wproj16_d.rearrange("(k p) f -> p k f", p=128))
        mask01 = cpool.tile([128, T], F16)
        nc.sync.dma_start(mask01, mask01_d)
        ident16 = cpool.tile([128, 128], F16)
        nc.sync.dma_start(ident16, ident16_d)

        # persistent kz / vp (ping-pong): zero halves and ones cols written once
        kz_pp = [cpool.tile([128, H, GT], F16, name=f"kz{i}") for i in range(2)]
        vp_pp = [cpool.tile([128, GB, H, 65], F16, name=f"vp{i}") for i in range(2)]
        for kzt in kz_pp:
            nc.gpsimd.memset(kzt[64:128, 0:H:2, :], 0.0)
            nc.gpsimd.memset(kzt[0:64, 1:H:2, :], 0.0)
        for vpt in vp_pp:
            nc.gpsimd.memset(vpt[:, :, :, 64:65], 1.0)

        def emit_x_chain(g):
            """Load + transpose x (fp16 from host) for group g."""
            xT = gpool.tile([128, NK, GB, 128], F16, tag="xT", bufs=4,
                             name=f"xT_{g}")
            for b in range(GB):
                x_sb = gpool.tile([128, C], F16, tag="x_sb", bufs=8,
                                  name=f"x_sb_{g}_{b}")
                nc.gpsimd.dma_start(x_sb, x_d[g * GB + b])
                nc.sync.dma_start_transpose(xT[:, :, b, :], x_sb)
            return xT

        def emit_y(g, b, oT):
            """Y projection for batch b of group g, reading transposed oT."""
            y_sb = spool.tile([128, C], F16, tag="y_sb", bufs=4, name="y_sb")
            for half in range(2):
                y_ps = pp.tile([128, 384], F32, tag="vps", bufs=2, name="y_ps")
                for k in range(NK):
                    nc.tensor.matmul(y_ps, oT[:, k, :],
                                     wproj16[:, k, 384 * half:384 * (half + 1)],
                                     start=(k == 0), stop=(k == NK - 1))
                y_copy = (nc.vector.tensor_copy if y_eng == "dve"
                          else nc.scalar.copy)
                y_copy(y_sb[:, 384 * half:384 * (half + 1)], y_ps)
            nc.sync.dma_start(y_d[g * GB + b], y_sb)

        F_ORDER = (6, 0, 7, 1, 8, 2, 9, 3, 10, 4, 11, 5)

        def emit_qk_unit(qT, kz, xg, f):
            qk_ps = pp.tile([128, GT], F32, tag="big", bufs=4, name="qk_ps")
            for k in range(NK):
                nc.tensor.matmul(qk_ps, wqkv16[:, k, 128 * f:128 * (f + 1)],
                                 xg[:, k, :], start=(k == 0), stop=(k == NK - 1))
            if f < 6:
                nc.scalar.copy(qT[:, f, :], qk_ps)
            else:
                h0 = 2 * (f - 6)
                # split the half-copies between DVE and Act
                use_dve = kz_eng == "dve" or (kz_eng == "split" and f % 2 == 0)
                if use_dve:
                    nc.vector.tensor_copy(kz[0:64, h0, :], qk_ps[0:64, :])
                    nc.vector.tensor_copy(kz[64:128, h0 + 1, :],
                                          qk_ps[64:128, :])
                else:
                    nc.scalar.copy(kz[0:64, h0, :], qk_ps[0:64, :])
                    nc.scalar.copy(kz[64:128, h0 + 1, :], qk_ps[64:128, :])

        def emit_v_unit(vp, xT, unit):
            b, half = divmod(unit, 2)
            v_ps = pp.tile([128, 384], F32, tag="vps", bufs=2, name="v_ps")
            for k in range(NK):
                nc.tensor.matmul(
                    v_ps, xT[:, k, b, :],
                    wqkv16[:, k, 2 * C + 384 * half:2 * C + 384 * (half + 1)],
                    start=(k == 0), stop=(k == NK - 1))
            vp_copy = (nc.scalar.copy if vp_eng == "act"
                       else nc.vector.tensor_copy)
            vp_copy(vp[:, b, 6 * half:6 * (half + 1), 0:64],
                    v_ps.rearrange("p (h d) -> p h d", d=64))

        def new_qT(g):
            return gpool.tile([128, 6, GT], F16, tag="qT", bufs=4,
                              name=f"qT_{g}")

        # prologue: first group's x-chain (steady state comes from the loop tail)
        xT_next = emit_x_chain(0)
        if flat:
            # flat mode: qk/V of group g+1 are interleaved into group g's
            # attention pipeline, so prefetch runs two groups ahead.
            xT_cur = xT_next
            xT_next = emit_x_chain(1)
            qT_next = new_qT(0)
            xg0 = xT_cur.rearrange("p k b t -> p k (b t)")
            for f in F_ORDER:
                emit_qk_unit(qT_next, kz_pp[0], xg0, f)
            for unit in range(2 * GB):
                emit_v_unit(vp_pp[0], xT_cur, unit)

        loop_cm = tc.For_i(0, loop_iters, 1) if loop_iters else nullcontext()
        with loop_cm:
          for _rep in range(py_iters):
            for g in range(NG):
              if flat:
                # qT/kz/vp for g were produced during g-1 (or the prologue);
                # xT_next holds x^T(g+1) for the interleaved qk/V below.
                qT, kz, vp = qT_next, kz_pp[g % 2], vp_pp[g % 2]
                xT_np1 = xT_next
                xT_next = emit_x_chain((g + 2) % NG)
                qT_next = new_qT((g + 1) % NG)
                kz_next = kz_pp[(g + 1) % 2]
                vp_next = vp_pp[(g + 1) % 2]
                xg_np1 = xT_np1.rearrange("p k b t -> p k (b t)")

                o_sb = gpool.tile([128, GB, C], F16, tag="o_sb", name=f"o_sb_{g}")
                eTs = [None] * 12
                oTs = [None] * GB
                for step in range(max(12 + 2 * GB, 12 + lag + lag_y + 1)):
                    if step < 12:
                        emit_qk_unit(qT_next, kz_next, xg_np1, F_ORDER[step])
                    v = step - lag
                    if 0 <= v < 12:
                        b, dpr = divmod(v, 3)
                        h0 = 4 * dpr
                        eT = eTs[v]
                        op_ps = pp.tile([128, 4, 65], F32, tag="op", bufs=2,
                                        name="op_ps")
                        for j in range(4):
                            nc.tensor.matmul(op_ps[:, j, :], eT[:, j, :],
                                             vp[:, b, h0 + j, :],
                                             start=(j == 0), stop=(j == 3))
                        rinv = spool.tile([128, 4], F32, tag="rinv", name="rinv")
                        nc.vector.reciprocal(rinv, op_ps[:, :, 64])
                        nc.vector.tensor_tensor(
                            out=o_sb[:, b, h0 * D:(h0 + 4) * D].rearrange(
                                "p (h d) -> p h d", h=4),
                            in0=op_ps[:, :, 0:64],
                            in1=rinv[:, :, None].broadcast_to([128, 4, 64]),
                            op=mybir.AluOpType.mult)
                        if dpr == 2:
                            oT = spool.tile([128, NK, 128], F16, tag="oT",
                                            bufs=4, name="oT")
                            if ot_xbar:
                                nc.sync.dma_start_transpose(oT, o_sb[:, b, :])
                            else:
                                for hf in range(2):
                                    ot_ps = pp.tile([128, 3, 128], F16,
                                                    tag="op", bufs=2,
                                                    name="ot_ps")
                                    for k in range(3):
                                        kk = 3 * hf + k
                                        nc.tensor.transpose(
                                            ot_ps[:, k, :],
                                            o_sb[:, b, 128 * kk:128 * (kk + 1)],
                                            ident16)
                                    nc.vector.tensor_copy(
                                        oT[:, 3 * hf:3 * hf + 3, :], ot_ps)
                            oTs[b] = oT
                    if step < 12:
                        b, dpr = divmod(step, 3)
                        h0 = 4 * dpr
                        bs = slice(b * T, (b + 1) * T)
                        st_ps = pp.tile([128, 4, T], F32, tag="big", bufs=4,
                                        name="st_ps")
                        for j in range(4):
                            nc.tensor.matmul(st_ps[:, j, :], kz[:, h0 + j, bs],
                                             qT[:, 2 * dpr + j // 2, bs],
                                             start=(j == 0), stop=(j == 3))
                        e_raw = spool.tile([128, 4, T], F16, tag="e_raw",
                                           bufs=eraw_bufs, name="e_raw")
                        nc.scalar.activation(e_raw, st_ps,
                                             mybir.ActivationFunctionType.Exp,
                                             scale=SCALE)
                        eT = spool.tile([128, 4, T], F16, tag="eT",
                                        bufs=et_bufs, name="eT")
                        nc.vector.tensor_tensor(
                            out=eT, in0=e_raw,
                            in1=mask01[:, None, :].broadcast_to([128, 4, T]),
                            op=mybir.AluOpType.mult)
                        eTs[step] = eT
                    if 12 <= step < 12 + 2 * GB:
                        emit_v_unit(vp_next, xT_np1, step - 12)
                    w = step - lag - lag_y
                    if 0 <= w < 12 and w % 3 == 2:
                        emit_y(g, w // 3, oTs[w // 3])
                continue
              else:
                xT = xT_next
                # prefetch next group's x -> fp16 -> x^T (wraps to g=0 for the
                # next loop iteration; reloads the same data, which is benign)
                xT_next = emit_x_chain((g + 1) % NG)

                # ---- Q^T / K^T projection (feature-major, fp16, N=512) ----
                qT = new_qT(g)
                kz = kz_pp[g % 2]
                xg = xT.rearrange("p k b t -> p k (b t)")
                # interleave K (f>=6) and Q (f<6) so attention can start early
                for f in F_ORDER:
                    emit_qk_unit(qT, kz, xg, f)

                # ---- V projection (token-major, fp16, N=384) into V' ----
                vp = vp_pp[g % 2]
                for unit in range(2 * GB):
                    emit_v_unit(vp, xT, unit)

                # ---- attention + output, software-pipelined (12 units) ----
                o_sb = gpool.tile([128, GB, C], F16, tag="o_sb", name=f"o_sb_{g}")
                eTs = [None] * 12
                oTs = [None] * GB
                for step in range(12 + lag + lag_y + 1):
                    v = step - lag
                    if 0 <= v < 12:
                        # EV first: frees the eT ring slot the mask below reuses
                        b, dpr = divmod(v, 3)
                        h0 = 4 * dpr
                        eT = eTs[v]
                        op_ps = pp.tile([128, 4, 65], F32, tag="op", bufs=2,
                                        name="op_ps")
                        for j in range(4):
                            nc.tensor.matmul(op_ps[:, j, :],
                                             eT[:, j, :],
                                             vp[:, b, h0 + j, :],
                                             start=(j == 0), stop=(j == 3))
                        rinv = spool.tile([128, 4], F32, tag="rinv", name="rinv")
                        nc.vector.reciprocal(rinv, op_ps[:, :, 64])
                        nc.vector.tensor_tensor(
                            out=o_sb[:, b, h0 * D:(h0 + 4) * D].rearrange(
                                "p (h d) -> p h d", h=4),
                            in0=op_ps[:, :, 0:64],
                            in1=rinv[:, :, None].broadcast_to([128, 4, 64]),
                            op=mybir.AluOpType.mult)
                        if dpr == 2:
                            oT = spool.tile([128, NK, 128], F16, tag="oT",
                                            bufs=4, name="oT")
                            if ot_xbar:
                                nc.sync.dma_start_transpose(oT, o_sb[:, b, :])
                            else:
                                # O^T via PE transposes (short latency)
                                for hf in range(2):
                                    ot_ps = pp.tile([128, 3, 128], F16, tag="op",
                                                    bufs=2, name="ot_ps")
                                    for k in range(3):
                                        kk = 3 * hf + k
                                        nc.tensor.transpose(
                                            ot_ps[:, k, :],
                                            o_sb[:, b, 128 * kk:128 * (kk + 1)],
                                            ident16)
                                    nc.vector.tensor_copy(
                                        oT[:, 3 * hf:3 * hf + 3, :], ot_ps)
                            oTs[b] = oT
                    if step < 12:
                        b, dpr = divmod(step, 3)
                        h0 = 4 * dpr
                        bs = slice(b * T, (b + 1) * T)
                        st_ps = pp.tile([128, 4, T], F32, tag="big", bufs=4,
                                        name="st_ps")
                        for j in range(4):
                            nc.tensor.matmul(st_ps[:, j, :], kz[:, h0 + j, bs],
                                             qT[:, 2 * dpr + j // 2, bs],
                                             start=(j == 0), stop=(j == 3))
                        e_raw = spool.tile([128, 4, T], F16, tag="e_raw",
                                           bufs=eraw_bufs, name="e_raw")
                        nc.scalar.activation(e_raw, st_ps,
                                             mybir.ActivationFunctionType.Exp,
                                             scale=SCALE)
                        eT = spool.tile([128, 4, T], F16, tag="eT",
                                        bufs=et_bufs, name="eT")
                        nc.vector.tensor_tensor(
                            out=eT, in0=e_raw,
                            in1=mask01[:, None, :].broadcast_to([128, 4, T]),
                            op=mybir.AluOpType.mult)
                        eTs[step] = eT
                    # Y once the oT copies have had LAG_Y units to land
                    w = step - lag - lag_y
                    if 0 <= w < 12 and w % 3 == 2:
                        bb = w // 3
                        emit_y(g, bb, oTs[bb])

    nc.compile()
    return nc


_PROGRAM = None
_in_maps_cache = None


def _host_consts():
    # S^T layout: partition = key, column = query -> keep (key <= query)
    mask01 = np.where(np.arange(T)[None, :] >= np.arange(128)[:, None],
                      np.float16(1.0), np.float16(0.0)).astype(np.float16)
    ident16 = np.eye(128, dtype=np.float16)
    return mask01, ident16


def make_in_maps(x, w_qkv, w_proj):
    x16 = np.ascontiguousarray(np.asarray(x), dtype=np.float16)
    w_qkv16 = np.ascontiguousarray(np.asarray(w_qkv), dtype=np.float16)
    w_proj16 = np.ascontiguousarray(np.asarray(w_proj), dtype=np.float16)
    mask01, ident16 = _host_consts()
    in_maps = []
    for c in range(N_CORES):
        in_maps.append({
            "x16": x16[c * BC:(c + 1) * BC],
            "w_qkv16": w_qkv16,
            "w_proj16": w_proj16,
            "mask01": mask01,
            "ident16": ident16,
        })
    return in_maps


def kernel(x, w_qkv, w_proj):
    global _PROGRAM, _in_maps_cache
    if _PROGRAM is None:
        _PROGRAM = build_program(**BEST)
    nc = _PROGRAM
    in_maps = make_in_maps(x, w_qkv, w_proj)
    _in_maps_cache = in_maps
    res = bass_utils.run_bass_kernel_spmd(nc, in_maps, core_ids=list(range(N_CORES)))
    out = np.concatenate([r["y"] for r in res.results], axis=0)
    return out.astype(np.float32)
